# revision 3
# baseline (speedup 1.0000x reference)
"""Data-parallel spatial-attention kernel for 8 Trainium2 NeuronCores.

Reference computation (per sample b):
  q = w1 . x (1x1 conv) + b1                 [1,H,W]
  k = w2 . x + b2                            [1,H,W]
  v = w3 . x + b3                            [C,H,W]
  scores[i,j] = sum_w q[i,w] k[j,w]          [H,H]
  attn = softmax(scores, axis=-1)
  out[c,i,w] = sum_j attn[i,j] v[c,j,w]      [C,H,W]

Sharding: batch B=64 split 8 ways (8 samples per core), weights replicated;
each sample's attention map is independent so no cross-core communication.
The three 1x1-conv projections are fused into one [C+2, C] weight so x is
read once instead of three times per core.
"""
import numpy as np
import jax
import jax.numpy as jnp

B, C, H, W = 64, 8, 256, 256
N_CORES = 8

_kernel_fn = None


def _local_attn(x, wall, ball):
    # wall: [C+2, C] rows stacked [w1; w2; w3]; ball: [C+2]
    qkv = jnp.einsum('bchw,oc->bohw', x, wall) + ball[None, :, None, None]
    q = qkv[:, 0]                # [Bl,H,W]
    k = qkv[:, 1]                # [Bl,H,W]
    v = qkv[:, 2:]               # [Bl,C,H,W]
    scores = jnp.einsum('bhw,bgw->bhg', q, k)
    attn = jax.nn.softmax(scores, axis=-1)
    out = jnp.einsum('bhg,bcgw->bchw', attn, v)
    return out


def _get_fn():
    global _kernel_fn
    if _kernel_fn is None:
        if len(jax.devices()) >= N_CORES:
            pfn = jax.pmap(_local_attn, in_axes=(0, None, None))
            _kernel_fn = lambda xs, w, b: pfn(xs, w, b)
        else:
            # fallback if the grading process exposes <8 devices
            jfn = jax.jit(_local_attn)
            _kernel_fn = lambda xs, w, b: jfn(
                xs.reshape(B, C, H, W), w, b).reshape(xs.shape[0],
                                                      xs.shape[1], C, H, W)
    return _kernel_fn


def kernel(x, w1, b1, w2, b2, w3, b3):
    x = np.asarray(x, dtype=np.float32)
    xs = x.reshape(N_CORES, B // N_CORES, C, H, W)
    wall = np.concatenate([np.asarray(w1, np.float32),
                           np.asarray(w2, np.float32),
                           np.asarray(w3, np.float32)], axis=0)
    ball = np.concatenate([np.asarray(b1, np.float32),
                           np.asarray(b2, np.float32),
                           np.asarray(b3, np.float32)], axis=0)
    out = _get_fn()(xs, wall, ball)
    return np.asarray(out, dtype=np.float32).reshape(B, C, H, W)
